# revision 1
# baseline (speedup 1.0000x reference)
"""GAT (2-layer, PyG-style) on 8 Trainium2 NeuronCores.

Strategy (edge-parallel, dst-sharded):
  - Host adds self-loops, sorts edges by dst, assigns dst-ranges of 6250
    nodes to each of 8 cores, tiles each core's nodes into 128-node groups,
    and chunks each group's edges into 128-edge chunks (split by src<32768
    parity because dma_gather indices are int16).
  - Device (per core): for each chunk, gather x[src] (bf16, transposed) and
    recompute h_src = x_src @ W1 on the PE; attention logits via an extra
    8-col matmul (als) plus a gathered per-node a_dst table (ald); segment
    softmax is folded into selection-matrix matmuls accumulating
    num = sum(exp*h) and den = sum(exp) per 128-node group in PSUM (no
    explicit alpha normalization, no scatter).  Layer 2 gathers rows of a
    small AllGathered g2=[h2@W2 | als2 | ald2] table.  Mean-pool partials
    are computed with one more selection matmul and AllReduced; the final
    fc + log_softmax runs replicated on every core.
"""

import os
import sys

sys.path.insert(0, "/opt/trn_rl_repo")

import numpy as np
import ml_dtypes

BF16 = ml_dtypes.bfloat16

# problem constants (hardcoded per contract)
N = 50000
E0 = 400000
F = 128
HID = 64
H1 = 8
HC = 512  # H1*HID
G = 64
CLS = 10
SLOPE = 0.2
NCORES = 8
NPC = N // NCORES  # 6250
NT = (NPC + 127) // 128  # 49
NPAD = NT * 128  # 6272
SPLIT = 32768
CB = 32  # chunks per gather batch
EB = CB  # chunks per psumE bank (one bank's als group == one d-batch)


def _set_size(n, e0, split, cb):
    """Debug helper: shrink the problem for simulator runs."""
    global N, E0, NPC, NT, NPAD, SPLIT, CB, EB
    N, E0, SPLIT, CB = n, e0, split, cb
    NPC = N // NCORES
    NT = (NPC + 127) // 128
    NPAD = NT * 128
    EB = CB


def _wrap_idx(idx):
    """[M] int -> [128, M//16] int16 in the dma_gather wrapped layout."""
    M = len(idx)
    assert M % 16 == 0
    a = np.asarray(idx, dtype=np.int16).reshape(M // 16, 16).T  # [16, M/16]
    return np.tile(a, (8, 1)).copy()  # [128, M/16]


def preprocess(edge_index, batch):
    """Build the shared chunk schedule plus per-core index/side arrays."""
    src = np.concatenate([edge_index[0], np.arange(N, dtype=np.int64)])
    dst = np.concatenate([edge_index[1], np.arange(N, dtype=np.int64)])
    order = np.argsort(dst, kind="stable")
    src, dst = src[order], dst[order]

    # bucket[core][group][parity] -> (src_list, dst_list)
    buckets = [[[None, None] for _ in range(NT)] for _ in range(NCORES)]
    core_of = dst // NPC
    for k in range(NCORES):
        m = core_of == k
        s_k, d_k = src[m], dst[m]
        dloc = d_k - NPC * k
        g_k = dloc // 128
        p_k = (s_k >= SPLIT).astype(np.int64)
        keys = g_k * 2 + p_k
        o2 = np.argsort(keys, kind="stable")
        s_k, d_k, keys = s_k[o2], d_k[o2], keys[o2]
        bounds = np.searchsorted(keys, np.arange(2 * NT + 1))
        for g in range(NT):
            for p in range(2):
                lo, hi = bounds[2 * g + p], bounds[2 * g + p + 1]
                buckets[k][g][p] = (s_k[lo:hi], d_k[lo:hi])

    # shared chunk counts
    nch = np.zeros((NT, 2), dtype=np.int64)
    for g in range(NT):
        for p in range(2):
            mx = max(len(buckets[k][g][p][0]) for k in range(NCORES))
            nch[g, p] = (mx + 127) // 128

    # shared schedule
    chunks = []  # dicts: g, p, sslot, c, first, last
    scount = [0, 0]
    for g in range(NT):
        first_c = len(chunks)
        for p in range(2):
            for _ in range(nch[g, p]):
                chunks.append(
                    dict(g=g, p=p, sslot=scount[p], c=len(chunks), first=False, last=False)
                )
                scount[p] += 1
        assert len(chunks) > first_c, f"group {g} has no chunks"
        chunks[first_c]["first"] = True
        chunks[-1]["last"] = True
    NCH = len(chunks)
    NSL, NSH = scount
    NBL = (NSL + CB - 1) // CB
    NBH = (NSH + CB - 1) // CB
    NBD = (NCH + CB - 1) // CB

    # runs: maximal consecutive chunk spans, same parity, same group, not
    # crossing CB (d-batch) or src-batch or EB boundaries
    runs = []  # (c0, r, p, s0)
    i = 0
    while i < NCH:
        c0 = chunks[i]
        j = i + 1
        while (
            j < NCH
            and chunks[j]["p"] == c0["p"]
            and chunks[j]["g"] == c0["g"]
            and chunks[j]["c"] // CB == c0["c"] // CB
            and chunks[j]["c"] // EB == c0["c"] // EB
            and chunks[j]["sslot"] // CB == c0["sslot"] // CB
            and chunks[j]["sslot"] == c0["sslot"] + (j - i)
        ):
            j += 1
        runs.append((c0["c"], j - i, c0["p"], c0["sslot"]))
        i = j

    # per-core arrays
    per_core = []
    for k in range(NCORES):
        sidx = [np.zeros(NBL * CB * 128, np.int64) - 1, np.zeros(NBH * CB * 128, np.int64) - 1]
        didx = np.zeros(NBD * CB * 128, np.int64) - 1
        dstlocT = np.full((128, NBD * CB), -1.0, np.float32)
        for ch in chunks:
            g, p, ss, c = ch["g"], ch["p"], ch["sslot"], ch["c"]
            s_e, d_e = buckets[k][g][p]
            ne = len(s_e)
            sv = np.zeros(128, np.int64)
            dv = np.zeros(128, np.int64)
            dl = np.full(128, -1.0, np.float32)
            # position of this chunk among its (g,p) bucket's chunks
            jprev = ss - sum(nch[gg, p] for gg in range(g))
            lo = jprev * 128
            hi = min(lo + 128, ne)
            nval = max(0, hi - lo)
            if nval > 0:
                sv[:nval] = s_e[lo:hi]
                dv[:nval] = d_e[lo:hi]
                dl[:nval] = (d_e[lo:hi] - (NPC * k + 128 * g)).astype(np.float32)
            if p == 1:
                sv = np.where(sv >= SPLIT, sv - SPLIT, 0)
            sidx[p][ss * 128 : ss * 128 + 128] = sv
            didx[c * 128 : c * 128 + 128] = dv - NPC * k
            didx[c * 128 : c * 128 + 128] = np.where(
                didx[c * 128 : c * 128 + 128] >= 0, didx[c * 128 : c * 128 + 128], 0
            )
            dstlocT[:, c] = dl
        # pads beyond streams stay -1 (trailing only)
        gidT = np.full((128, NT), -1.0, np.float32)
        for t in range(NT):
            n0 = NPC * k + 128 * t
            nt_ = min(128, NPC * (k + 1) - n0)
            gidT[:nt_, t] = batch[n0 : n0 + nt_].astype(np.float32)
        per_core.append(
            dict(
                sidx_lo=_wrap_idx(sidx[0]),
                sidx_hi=_wrap_idx(sidx[1]),
                didx=_wrap_idx(didx),
                dstlocT=dstlocT,
                gidT=gidT,
            )
        )

    sched = dict(chunks=chunks, runs=runs, NCH=NCH, NSL=NSL, NSH=NSH, NBL=NBL, NBH=NBH, NBD=NBD)
    return sched, per_core


def build_program(sched):
    """Build the (shared) 8-core bass program for the given schedule."""
    import concourse.bass as bass
    import concourse.tile as tile
    from concourse import bacc, mybir

    f32 = mybir.dt.float32
    bf16 = mybir.dt.bfloat16
    i16 = mybir.dt.int16
    AF = mybir.ActivationFunctionType
    OP = mybir.AluOpType

    NCH, NBL, NBH, NBD = sched["NCH"], sched["NBL"], sched["NBH"], sched["NBD"]
    chunks, runs = sched["chunks"], sched["runs"]

    nc = bacc.Bacc(
        "TRN2",
        target_bir_lowering=False,
        debug=False,
        enable_asserts=False,
        num_swdge_queues=4,
        num_devices=NCORES,
    )

    # ---- I/O ----
    def din(name, shape, dt):
        return nc.dram_tensor(name, shape, dt, kind="ExternalInput")

    xlo = din("xlo", [SPLIT, F], bf16)
    xhi = din("xhi", [N - SPLIT, F], bf16)
    xTown = din("xTown", [F, NPC], f32)
    w1b = din("w1b", [F, HC], bf16)
    asb = din("asb", [F, H1], bf16)
    adf = din("adf", [F, H1], f32)
    w2e = din("w2e", [HC, HID + 2], bf16)
    fcwb = din("fcwb", [HID + 1, CLS], f32)
    sidx_lo = din("sidx_lo", [128, NBL * CB * 8], i16)
    sidx_hi = din("sidx_hi", [128, NBH * CB * 8], i16)
    didx = din("didx", [128, NBD * CB * 8], i16)
    dstlocT = din("dstlocT", [128, NBD * CB], f32)
    gidT = din("gidT", [128, NT], f32)
    out = nc.dram_tensor("out", [G, CLS], f32, kind="ExternalOutput")

    iota_np = np.tile(np.arange(128, dtype=np.float32), (128, 1))
    iota_dram = nc.inline_tensor(iota_np, name="iota128")

    # ---- internal DRAM ----
    ald_own = nc.dram_tensor("ald_own", [NPC, 64], f32)
    h2_dram = nc.dram_tensor("h2_dram", [NPAD, HC], bf16)
    g2_own = nc.dram_tensor("g2_own", [NPC, 128], bf16)
    g2_full = nc.dram_tensor("g2_full", [N, 128], bf16, addr_space="Shared")
    pool_own = nc.dram_tensor("pool_own", [HID + 1, G], f32)
    pool_ar = nc.dram_tensor("pool_ar", [HID + 1, G], f32, addr_space="Shared")
    pool_loc = nc.dram_tensor("pool_loc", [HID + 1, G], f32)

    RG = [list(range(NCORES))]

    with tile.TileContext(nc) as tc:
        with tc.tile_pool(name="const", bufs=1) as cpool:
            iota_sb = cpool.tile([128, 128], f32)
            nc.sync.dma_start(iota_sb[:], iota_dram[:])
            w1b_sb = cpool.tile([F, HC], bf16)
            nc.sync.dma_start(w1b_sb[:], w1b[:])
            asb_sb = cpool.tile([F, H1], bf16)
            nc.sync.dma_start(asb_sb[:], asb[:])
            adf_sb = cpool.tile([F, H1], f32)
            nc.sync.dma_start(adf_sb[:], adf[:])
            gid_sb = cpool.tile([128, NT], f32)
            nc.sync.dma_start(gid_sb[:], gidT[:])

            PHASES = os.environ.get("GAT_PHASES", "ABCDE")
            # ---- phase A: ald_own = x_own @ A_d  (f32) ----
            with (
                tc.tile_pool(name="pa_sbuf", bufs=3) as pa,
                tc.tile_pool(name="pa_psum", bufs=2, space="PSUM") as pap,
            ):
                for t in range(NT):
                    nt_ = min(128, NPC - 128 * t)
                    xt = pa.tile([F, 128], f32, tag="xt")
                    nc.sync.dma_start(xt[:, :nt_], xTown[:, 128 * t : 128 * t + nt_])
                    ps = pap.tile([128, H1], f32)
                    nc.tensor.matmul(
                        out=ps[:nt_, :], lhsT=xt[:, :nt_], rhs=adf_sb[:], start=True, stop=True
                    )
                    av = pa.tile([128, H1], f32, tag="av")
                    nc.vector.tensor_copy(av[:nt_, :], ps[:nt_, :])
                    nc.sync.dma_start(
                        ald_own[128 * t : 128 * t + nt_, 0:H1], av[:nt_, :]
                    )

            # ---- phase B: layer-1 edge processing ----
            if "B" in PHASES:
                with (
                    tc.tile_pool(name="gx", bufs=2) as gxp,
                    tc.tile_pool(name="gd", bufs=2) as gdp,
                    tc.tile_pool(name="gi", bufs=2) as gip,
                    tc.tile_pool(name="hsb", bufs=CB + 4) as hsp,
                    tc.tile_pool(name="sS", bufs=2) as ssp,
                    tc.tile_pool(name="sE", bufs=2) as sep,
                    tc.tile_pool(name="msg", bufs=3) as msp,
                    tc.tile_pool(name="fin", bufs=2) as fip,
                    tc.tile_pool(name="psH", bufs=2, space="PSUM") as psH,
                    tc.tile_pool(name="psN", bufs=2, space="PSUM") as psN,
                    tc.tile_pool(name="psE", bufs=2, space="PSUM") as psE,
                    tc.tile_pool(name="psD", bufs=2, space="PSUM") as psD,
                ):
                    xbufs = {}  # (p, batch) -> tile
                    dbufs = {}
                    ebanks = {}
                    Sbuf = {}
                    hs = {}
                    psums = {}  # g -> (psumN, psumD)

                    def issue_src_batch(p, b):
                        nb = [NBL, NBH][p]
                        assert b < nb
                        tname = [sidx_lo, sidx_hi][p]
                        table = [xlo, xhi][p]
                        it = gip.tile([128, CB * 8], i16, tag=f"si{p}")
                        nc.sync.dma_start(it[:], tname[:, b * CB * 8 : (b + 1) * CB * 8])
                        xb = gxp.tile([128, 1, CB * 128], bf16, tag=f"x{p}")
                        nsl = [sched["NSL"], sched["NSH"]][p]
                        nval = min(CB, nsl - b * CB) * 128
                        nc.gpsimd.dma_gather(
                            out_ap=xb[:],
                            in_ap=table[:],
                            idxs_ap=it[:],
                            num_idxs=CB * 128,
                            num_idxs_reg=nval,
                            elem_size=F,
                            transpose=True,
                            single_packet=False,
                            queue_num=0,
                        )
                        xbufs[(p, b)] = xb

                    def issue_dst_batch(b):
                        it = gip.tile([128, CB * 8], i16, tag="di")
                        nc.sync.dma_start(it[:], didx[:, b * CB * 8 : (b + 1) * CB * 8])
                        db = gdp.tile([128, CB, 64], f32, tag="d1")
                        nval = min(CB, NCH - b * CB) * 128
                        nc.gpsimd.dma_gather(
                            out_ap=db[:],
                            in_ap=ald_own[:],
                            idxs_ap=it[:],
                            num_idxs=CB * 128,
                            num_idxs_reg=nval,
                            elem_size=64,
                            transpose=False,
                            single_packet=False,
                            queue_num=1,
                        )
                        dbufs[b] = db
                        dl = gip.tile([128, CB], f32, tag="dl")
                        nc.sync.dma_start(dl[:], dstlocT[:, b * CB : b * CB + CB])
                        # build S for the batch (bf16 0/1)
                        S = ssp.tile([128, CB * 128], bf16, tag="S")
                        nc.vector.tensor_tensor(
                            out=S[:].rearrange("p (a n) -> p a n", a=CB),
                            in0=dl[:].to_broadcast([128, CB, 128]),
                            in1=iota_sb[:]
                            .rearrange("p (a n) -> p a n", a=1)
                            .broadcast_to([128, CB, 128]),
                            op=OP.is_equal,
                        )
                        Sbuf[b] = S

                    # main chunk loop
                    for ch in chunks:
                        c, g, p, ss = ch["c"], ch["g"], ch["p"], ch["sslot"]
                        bs, js = ss // CB, ss % CB
                        bd, jd = c // CB, c % CB
                        eb = c // EB
                        if (p, bs) not in xbufs:
                            issue_src_batch(p, bs)
                        if bd not in dbufs:
                            issue_dst_batch(bd)
                        if eb not in ebanks:
                            ebanks[eb] = psE.tile([128, 512], f32, tag="E", name=f"E{eb}")
                        xs = xbufs[(p, bs)][:, 0, js * 128 : (js + 1) * 128]
                        # h_src and als
                        psh = psH.tile([128, HC], f32, tag="H")
                        nc.tensor.matmul(out=psh[:], lhsT=xs, rhs=w1b_sb[:], start=True, stop=True)
                        nc.tensor.matmul(
                            out=ebanks[eb][:, (c % EB) * 8 : (c % EB) * 8 + 8],
                            lhsT=xs,
                            rhs=asb_sb[:],
                            start=(c % EB == 0),
                            stop=(c % EB == EB - 1 or c == NCH - 1),
                        )
                        # copy h to sbuf (ACT) as bf16
                        h = hsp.tile([128, HC], bf16, tag="h")
                        nc.scalar.activation(h[:], psh[:], AF.Copy)
                        hs[c] = h

                        # once we hit the last chunk of a d-batch, run the exp path
                        if jd == CB - 1 or c == NCH - 1:
                            er = sep.tile([128, CB, H1], f32, tag="er")
                            for (c0, r, rp, s0) in runs:
                                if c0 // CB != bd:
                                    continue
                                ebk = ebanks[c0 // EB]
                                nc.vector.tensor_tensor(
                                    out=er[:, c0 % CB : c0 % CB + r, :],
                                    in0=ebk[
                                        :, (c0 % EB) * 8 : (c0 % EB) * 8 + 8 * r
                                    ].rearrange("p (a n) -> p a n", a=r),
                                    in1=dbufs[bd][:, c0 % CB : c0 % CB + r, 0:H1],
                                    op=OP.add,
                                )
                            elk = sep.tile([128, CB, H1], f32, tag="elk")
                            nc.vector.scalar_tensor_tensor(
                                out=elk[:],
                                in0=er[:],
                                scalar=SLOPE,
                                in1=er[:],
                                op0=OP.mult,
                                op1=OP.max,
                            )
                            ex = sep.tile([128, CB, H1], bf16, tag="ex")
                            nc.scalar.activation(
                                ex[:].rearrange("p a n -> p (a n)"),
                                elk[:].rearrange("p a n -> p (a n)"),
                                AF.Exp,
                            )
                            # weight + accumulate all chunks of this batch
                            for cc in range(bd * CB, min((bd + 1) * CB, NCH)):
                                ch2 = chunks[cc]
                                if ch2["first"]:
                                    psums[ch2["g"]] = (
                                        psN.tile([128, HC], f32, tag="N", name=f"N{ch2['g']}"),
                                        psD.tile([128, H1], f32, tag="D", name=f"D{ch2['g']}"),
                                    )
                                psumN, psumD = psums[ch2["g"]]
                                m = msp.tile([128, HC], bf16, tag="m")
                                nc.vector.tensor_tensor(
                                    out=m[:].rearrange("p (h k) -> p h k", h=H1),
                                    in0=hs[cc][:].rearrange("p (h k) -> p h k", h=H1),
                                    in1=ex[:, cc % CB, :]
                                    .rearrange("p (h o) -> p h o", o=1)
                                    .broadcast_to([128, H1, HID]),
                                    op=OP.mult,
                                )
                                Ssl = Sbuf[bd][:, (cc % CB) * 128 : (cc % CB) * 128 + 128]
                                nc.tensor.matmul(
                                    out=psumN[:],
                                    lhsT=Ssl,
                                    rhs=m[:],
                                    start=ch2["first"],
                                    stop=ch2["last"],
                                )
                                nc.tensor.matmul(
                                    out=psumD[:],
                                    lhsT=Ssl,
                                    rhs=ex[:, cc % CB, :],
                                    start=ch2["first"],
                                    stop=ch2["last"],
                                )
                                del hs[cc]
                                if ch2["last"]:
                                    # finalize group
                                    gg = ch2["g"]
                                    dd = fip.tile([128, H1], f32, tag="dd")
                                    nc.vector.tensor_scalar_add(dd[:], psumD[:], 1e-16)
                                    rc = fip.tile([128, H1], f32, tag="rc")
                                    nc.vector.reciprocal(rc[:], dd[:])
                                    o1 = fip.tile([128, HC], f32, tag="o1")
                                    nc.vector.tensor_tensor(
                                        out=o1[:].rearrange("p (h k) -> p h k", h=H1),
                                        in0=psumN[:].rearrange("p (h k) -> p h k", h=H1),
                                        in1=rc[:]
                                        .rearrange("p (h o) -> p h o", o=1)
                                        .broadcast_to([128, H1, HID]),
                                        op=OP.mult,
                                    )
                                    # elu = min(exp(x)-1, relu(x))
                                    expo = fip.tile([128, HC], f32, tag="expo")
                                    nc.scalar.activation(expo[:], o1[:], AF.Exp)
                                    rel = fip.tile([128, HC], f32, tag="rel")
                                    nc.vector.tensor_scalar_max(rel[:], o1[:], 0.0)
                                    h2t = fip.tile([128, HC], bf16, tag="h2t")
                                    nc.vector.scalar_tensor_tensor(
                                        out=h2t[:],
                                        in0=expo[:],
                                        scalar=-1.0,
                                        in1=rel[:],
                                        op0=OP.add,
                                        op1=OP.min,
                                    )
                                    nc.sync.dma_start(
                                        h2_dram[128 * gg : 128 * (gg + 1), :], h2t[:]
                                    )

            # ---- phase C: g2 table + AllGather ----
            if "C" in PHASES:
                with (
                    tc.tile_pool(name="pc_s", bufs=3) as pc,
                    tc.tile_pool(name="pc_h2t", bufs=1) as ph2,
                    tc.tile_pool(name="pc_ps", bufs=2, space="PSUM") as pcp,
                ):
                    w2_sb = pc.tile([128, 4, HID + 2], bf16, tag="w2")
                    for i in range(4):
                        nc.sync.dma_start(w2_sb[:, i, :], w2e[128 * i : 128 * (i + 1), :])
                    h2T = ph2.tile([128, 4, NPAD], bf16)
                    for i in range(4):
                        nc.sync.dma_start(
                            h2T[:, i, :],
                            h2_dram[:, 128 * i : 128 * (i + 1)],
                            transpose=True,
                        )
                    for t in range(NT):
                        nt_ = min(128, NPC - 128 * t)
                        ps = pcp.tile([128, HID + 2], f32)
                        for i in range(4):
                            nc.tensor.matmul(
                                out=ps[:],
                                lhsT=h2T[:, i, 128 * t : 128 * t + 128],
                                rhs=w2_sb[:, i, :],
                                start=(i == 0),
                                stop=(i == 3),
                            )
                        gv = pc.tile([128, HID + 2], bf16, tag="gv")
                        nc.vector.tensor_copy(gv[:nt_, :], ps[:nt_, :])
                        nc.sync.dma_start(
                            g2_own[128 * t : 128 * t + nt_, 0 : HID + 2], gv[:nt_, :]
                        )
                    nc.gpsimd.collective_compute(
                        "AllGather",
                        mybir.AluOpType.bypass,
                        replica_groups=RG,
                        ins=[g2_own[:]],
                        outs=[g2_full[:]],
                    )

            # ---- phase D: layer-2 edge processing + pooling ----
            if "D" in PHASES:
                with (
                    tc.tile_pool(name="g2x", bufs=2) as g2xp,
                    tc.tile_pool(name="g2d", bufs=2) as g2dp,
                    tc.tile_pool(name="gi2", bufs=2) as gip2,
                    tc.tile_pool(name="sS2", bufs=2) as ssp2,
                    tc.tile_pool(name="sE2", bufs=2) as sep2,
                    tc.tile_pool(name="m2", bufs=2) as msp2,
                    tc.tile_pool(name="fin2", bufs=2) as fip2,
                    tc.tile_pool(name="psN2", bufs=2, space="PSUM") as psN2,
                    tc.tile_pool(name="psP", bufs=1, space="PSUM") as psP,
                ):
                    xbufs2 = {}
                    dbufs2 = {}
                    Sbuf2 = {}
                    psums2 = {}
                    psumPool = psP.tile([HID + 1, G], f32)
                    ones_col = cpool.tile([128, 1], bf16)
                    nc.gpsimd.memset(ones_col[:], 1.0)

                    def issue_src2(p, b):
                        tname = [sidx_lo, sidx_hi][p]
                        it = gip2.tile([128, CB * 8], i16, tag=f"si{p}")
                        nc.sync.dma_start(it[:], tname[:, b * CB * 8 : (b + 1) * CB * 8])
                        xb = g2xp.tile([128, CB, 128], bf16, tag=f"x{p}")
                        nsl = [sched["NSL"], sched["NSH"]][p]
                        nval = min(CB, nsl - b * CB) * 128
                        table = g2_full[0:SPLIT, :] if p == 0 else g2_full[SPLIT:N, :]
                        nc.gpsimd.dma_gather(
                            out_ap=xb[:],
                            in_ap=table,
                            idxs_ap=it[:],
                            num_idxs=CB * 128,
                            num_idxs_reg=nval,
                            elem_size=128,
                            transpose=False,
                            single_packet=False,
                            queue_num=2,
                        )
                        xbufs2[(p, b)] = xb

                    def issue_dst2(b):
                        it = gip2.tile([128, CB * 8], i16, tag="di")
                        nc.sync.dma_start(it[:], didx[:, b * CB * 8 : (b + 1) * CB * 8])
                        db = g2dp.tile([128, CB, 128], bf16, tag="d2")
                        nval = min(CB, NCH - b * CB) * 128
                        nc.gpsimd.dma_gather(
                            out_ap=db[:],
                            in_ap=g2_own[:],
                            idxs_ap=it[:],
                            num_idxs=CB * 128,
                            num_idxs_reg=nval,
                            elem_size=128,
                            transpose=False,
                            single_packet=False,
                            queue_num=3,
                        )
                        dbufs2[b] = db
                        dl = gip2.tile([128, CB], f32, tag="dl")
                        nc.sync.dma_start(dl[:], dstlocT[:, b * CB : b * CB + CB])
                        S = ssp2.tile([128, CB * 128], bf16, tag="S")
                        nc.vector.tensor_tensor(
                            out=S[:].rearrange("p (a n) -> p a n", a=CB),
                            in0=dl[:].to_broadcast([128, CB, 128]),
                            in1=iota_sb[:]
                            .rearrange("p (a n) -> p a n", a=1)
                            .broadcast_to([128, CB, 128]),
                            op=OP.is_equal,
                        )
                        Sbuf2[b] = S

                    for ch in chunks:
                        c, g, p, ss = ch["c"], ch["g"], ch["p"], ch["sslot"]
                        bs, js = ss // CB, ss % CB
                        bd, jd = c // CB, c % CB
                        if (p, bs) not in xbufs2:
                            issue_src2(p, bs)
                        if bd not in dbufs2:
                            issue_dst2(bd)

                        if jd == CB - 1 or c == NCH - 1:
                            # batched attention for this d-batch
                            er = sep2.tile([128, CB], f32, tag="er")
                            for (c0, r, rp, s0) in runs:
                                if c0 // CB != bd:
                                    continue
                                nc.vector.tensor_tensor(
                                    out=er[:, c0 % CB : c0 % CB + r].rearrange(
                                        "p (a o) -> p a o", o=1
                                    ),
                                    in0=xbufs2[(rp, s0 // CB)][
                                        :, s0 % CB : s0 % CB + r, HID : HID + 1
                                    ],
                                    in1=dbufs2[bd][:, c0 % CB : c0 % CB + r, HID + 1 : HID + 2],
                                    op=OP.add,
                                )
                            elk = sep2.tile([128, CB], f32, tag="elk")
                            nc.vector.scalar_tensor_tensor(
                                out=elk[:],
                                in0=er[:],
                                scalar=SLOPE,
                                in1=er[:],
                                op0=OP.mult,
                                op1=OP.max,
                            )
                            ex = sep2.tile([128, CB], bf16, tag="ex")
                            nc.scalar.activation(ex[:], elk[:], AF.Exp)
                            me = msp2.tile([128, CB, HID + 1], bf16, tag="me")
                            for (c0, r, rp, s0) in runs:
                                if c0 // CB != bd:
                                    continue
                                nc.vector.tensor_tensor(
                                    out=me[:, c0 % CB : c0 % CB + r, 0:HID],
                                    in0=xbufs2[(rp, s0 // CB)][:, s0 % CB : s0 % CB + r, 0:HID],
                                    in1=ex[:, c0 % CB : c0 % CB + r]
                                    .rearrange("p (a o) -> p a o", o=1)
                                    .broadcast_to([128, r, HID]),
                                    op=OP.mult,
                                )
                            nc.vector.tensor_copy(
                                me[:, :, HID : HID + 1],
                                ex[:].rearrange("p (a o) -> p a o", o=1),
                            )
                            for cc in range(bd * CB, min((bd + 1) * CB, NCH)):
                                ch2 = chunks[cc]
                                if ch2["first"]:
                                    psums2[ch2["g"]] = psN2.tile(
                                        [128, HID + 1], f32, tag="N2", name=f"N2_{ch2['g']}"
                                    )
                                psumN2 = psums2[ch2["g"]]
                                Ssl = Sbuf2[bd][:, (cc % CB) * 128 : (cc % CB) * 128 + 128]
                                nc.tensor.matmul(
                                    out=psumN2[:],
                                    lhsT=Ssl,
                                    rhs=me[:, cc % CB, :],
                                    start=ch2["first"],
                                    stop=ch2["last"],
                                )
                                if ch2["last"]:
                                    gg = ch2["g"]
                                    nt_ = min(128, NPC - 128 * gg)
                                    dd = fip2.tile([128, 1], f32, tag="dd")
                                    nc.vector.tensor_scalar_add(
                                        dd[:], psumN2[:, HID : HID + 1], 1e-16
                                    )
                                    rc = fip2.tile([128, 1], f32, tag="rc")
                                    nc.vector.reciprocal(rc[:], dd[:])
                                    o2e = fip2.tile([128, HID + 1], bf16, tag="o2e")
                                    nc.vector.tensor_scalar(
                                        out=o2e[:, 0:HID],
                                        in0=psumN2[:, 0:HID],
                                        scalar1=rc[:],
                                        scalar2=None,
                                        op0=OP.mult,
                                    )
                                    nc.vector.tensor_copy(
                                        o2e[:, HID : HID + 1], ones_col[:]
                                    )
                                    gt = fip2.tile([128, G], bf16, tag="gt")
                                    nc.vector.tensor_tensor(
                                        out=gt[:],
                                        in0=gid_sb[:, gg : gg + 1].to_broadcast([128, G]),
                                        in1=iota_sb[:, 0:G],
                                        op=OP.is_equal,
                                    )
                                    nc.tensor.matmul(
                                        out=psumPool[:],
                                        lhsT=o2e[:],
                                        rhs=gt[:],
                                        start=(gg == 0),
                                        stop=(gg == NT - 1),
                                    )

                    # pool -> DRAM -> AllReduce
                    plsb = fip2.tile([HID + 1, G], f32, tag="pl")
                    nc.vector.tensor_copy(plsb[:], psumPool[:])
                    nc.sync.dma_start(pool_own[:], plsb[:])
                    nc.gpsimd.collective_compute(
                        "AllReduce",
                        mybir.AluOpType.add,
                        replica_groups=RG,
                        ins=[pool_own[:]],
                        outs=[pool_ar[:]],
                    )

            # ---- phase E: fc + log_softmax (replicated) ----
            if "E" in PHASES:
                with (
                    tc.tile_pool(name="pe_s", bufs=1) as pe,
                    tc.tile_pool(name="pe_ps", bufs=1, space="PSUM") as pep,
                ):
                    nc.sync.dma_start(pool_loc[:], pool_ar[:])
                    poolA = pe.tile([HID + 1, G], f32)
                    nc.sync.dma_start(poolA[:], pool_loc[:])
                    fcw_sb = pe.tile([HID + 1, CLS], f32)
                    nc.sync.dma_start(fcw_sb[:], fcwb[:])
                    cnt = pe.tile([G, 1], f32)
                    nc.sync.dma_start(cnt[:], pool_loc[HID : HID + 1, :].rearrange("a g -> g a"))
                    lg_ps = pep.tile([G, CLS], f32)
                    nc.tensor.matmul(
                        out=lg_ps[:], lhsT=poolA[:], rhs=fcw_sb[:], start=True, stop=True
                    )
                    cnt1 = pe.tile([G, 1], f32)
                    nc.vector.tensor_scalar_max(cnt1[:], cnt[:], 1.0)
                    rcnt = pe.tile([G, 1], f32)
                    nc.vector.reciprocal(rcnt[:], cnt1[:])
                    lg = pe.tile([G, CLS], f32)
                    nc.vector.tensor_scalar(
                        out=lg[:], in0=lg_ps[:], scalar1=rcnt[:], scalar2=None, op0=OP.mult
                    )
                    mx = pe.tile([G, 1], f32)
                    nc.vector.reduce_max(mx[:], lg[:], axis=mybir.AxisListType.X)
                    lgs = pe.tile([G, CLS], f32)
                    nc.vector.tensor_scalar(
                        out=lgs[:], in0=lg[:], scalar1=mx[:], scalar2=None, op0=OP.subtract
                    )
                    ex = pe.tile([G, CLS], f32)
                    sume = pe.tile([G, 1], f32)
                    nc.scalar.activation(ex[:], lgs[:], AF.Exp, accum_out=sume[:])
                    lse = pe.tile([G, 1], f32)
                    nc.scalar.activation(lse[:], sume[:], AF.Ln)
                    res = pe.tile([G, CLS], f32)
                    nc.vector.tensor_scalar(
                        out=res[:], in0=lgs[:], scalar1=lse[:], scalar2=None, op0=OP.subtract
                    )
                    nc.sync.dma_start(out[:], res[:])

    nc.compile()
    return nc


def make_inputs(x, edge_index, batch, W1, a_src1, a_dst1, b1, W2, a_src2, a_dst2, b2, fc_w, fc_b):
    """Host-side preprocessing -> (sched, in_maps)."""
    x = np.asarray(x, np.float32)
    edge_index = np.asarray(edge_index, np.int64)
    batch = np.asarray(batch, np.int64)
    W1 = np.asarray(W1, np.float32)
    a_src1 = np.asarray(a_src1, np.float32)
    a_dst1 = np.asarray(a_dst1, np.float32)
    W2 = np.asarray(W2, np.float32)
    a_src2 = np.asarray(a_src2, np.float32)
    a_dst2 = np.asarray(a_dst2, np.float32)
    fc_w = np.asarray(fc_w, np.float32)
    fc_b = np.asarray(fc_b, np.float32)
    b1 = np.asarray(b1, np.float32)
    b2 = np.asarray(b2, np.float32)
    assert not np.any(b1), "kernel assumes b1 == 0 (setup_inputs gives zeros)"

    sched, per_core = preprocess(edge_index, batch)

    W1r = W1.reshape(F, H1, HID)
    A_s = np.einsum("fhc,hc->fh", W1r, a_src1).astype(np.float32)
    A_d = np.einsum("fhc,hc->fh", W1r, a_dst1).astype(np.float32)
    w_as2 = (W2 @ a_src2[0]).astype(np.float32)
    w_ad2 = (W2 @ a_dst2[0]).astype(np.float32)
    w2e = np.concatenate([W2, w_as2[:, None], w_ad2[:, None]], axis=1)
    fc_b2 = fc_b + b2 @ fc_w
    fcwb = np.concatenate([fc_w, fc_b2[None, :]], axis=0).astype(np.float32)

    common = dict(
        xlo=x[:SPLIT].astype(BF16),
        xhi=x[SPLIT:].astype(BF16),
        w1b=W1.astype(BF16),
        asb=A_s.astype(BF16),
        adf=A_d,
        w2e=w2e.astype(BF16),
        fcwb=fcwb,
    )
    in_maps = []
    for k in range(NCORES):
        pc = per_core[k]
        m = dict(common)
        m["xTown"] = np.ascontiguousarray(x[NPC * k : NPC * (k + 1)].T)
        m["sidx_lo"] = pc["sidx_lo"]
        m["sidx_hi"] = pc["sidx_hi"]
        m["didx"] = pc["didx"]
        m["dstlocT"] = pc["dstlocT"]
        m["gidT"] = pc["gidT"]
        in_maps.append(m)
    return sched, in_maps


def kernel(**inputs):
    sched, in_maps = make_inputs(**inputs)
    nc = build_program(sched)
    from concourse.bass_utils import run_bass_kernel_spmd

    trace = bool(int(os.environ.get("GAT_TRACE", "0")))
    res = run_bass_kernel_spmd(
        nc, in_maps, core_ids=list(range(NCORES)), trace=trace
    )
    if trace and res.exec_time_ns is not None:
        print(f"HW exec time: {res.exec_time_ns} ns")
        kernel.last_exec_time_ns = res.exec_time_ns
    return np.asarray(res.results[0]["out"], np.float32)

